# revision 24
# baseline (speedup 1.0000x reference)
"""ConstituentAttention Trainium2 kernel.

Data-parallel over batch: core b handles batch b (B == 8 == n_cores).

Math (per batch, S=2048, E=1024, P=64):
  qkT[p, s] = sum_e W[p, e] x[s, e] + bias[p]         (p in [0,128): q rows 0..63, k rows 64..127)
  D_raw[s]  = sum_j q[j, s] * (k[j, s+1] - k[j, s-1])  == 1024*(score0[s] - score1[s])
  with D_raw[0] := +BIG, D_raw[2047] := -BIG (softmax -inf boundary handling),
  p0[s] = sigmoid(D[s]), shifted p1[s] = sigmoid(-D[s+1]), so
  g[s]  = 1 / ((1 + e^{-D[s]}) (1 + e^{+D[s+1]}))
  prob0 = sqrt(g + 1e-6) = exp(0.5*(ln(1e-6*P + 1) - ln(P))),  P = (1+e^-D)(1+e^+Dnext)
  neighbor = prior + prob0 - prior*prob0
  cs[t] = sum_{m<t} ln(neighbor[m])
  out[i, j] = exp(cs[max(i,j)] - cs[min(i,j)]),  0 on diag.
"""

import sys

sys.path.insert(0, "/opt/trn_rl_repo")

import ml_dtypes
import numpy as np

BFH = ml_dtypes.bfloat16

import concourse.bass as bass
import concourse.bacc as bacc
import concourse.tile as tile
from concourse import mybir
from concourse.bass_utils import run_bass_kernel_spmd
from concourse.masks import (
    make_identity,
    make_lower_triangular,
    make_upper_triangular,
)

F32 = mybir.dt.float32
F32R = mybir.dt.float32r
BF16 = mybir.dt.bfloat16
I32 = mybir.dt.int32
AF = mybir.ActivationFunctionType

S = 2048
E = 1024
B = 8
NCHUNK = 16  # S / 128
# Raw-score boundary magnitude; Exp scale 1/1024 maps it to +-30.
# 30 is chosen so 1+e^-30 == 1.0 exactly in f32 (p0[0] == 1) while
# P = (1+e^30)*2 ~ 4e13 stays inside ACT Ln's 2^64 domain and
# 1e-6*P >> 1 makes prob0[2047] -> sqrt(1e-6) as in the reference.
BIG = 30.0 * 1024.0


def build_nc(n_cores=8):
    nc = bacc.Bacc("TRN2", target_bir_lowering=False, debug=False, num_devices=n_cores, enable_partition_id=False)

    xb = nc.dram_tensor("xb", [S, E], BF16, kind="ExternalInput")
    w = nc.dram_tensor("w", [8, 128, 128], BF16, kind="ExternalInput")
    wbias = nc.dram_tensor("wbias", [128, 1], F32, kind="ExternalInput")
    priorb = nc.dram_tensor("priorb", [16, 128], F32, kind="ExternalInput")
    outc = nc.dram_tensor("outc", [S, S], F32, kind="ExternalOutput")
    outn = nc.dram_tensor("outn", [16, 128], F32, kind="ExternalOutput")

    with tile.TileContext(nc) as tc:
        build_tile_kernel(nc, tc, xb, w, wbias, priorb, outc, outn)
    nc.compile()
    return nc


def build_tile_kernel(nc, tc, xb, w, wbias, priorb, outc, outn):
    from contextlib import ExitStack

    ctx = ExitStack()
    with ctx:
        cpool = ctx.enter_context(tc.tile_pool(name="consts", bufs=1))
        xtpool = ctx.enter_context(tc.tile_pool(name="xt", bufs=2))
        spool = ctx.enter_context(tc.tile_pool(name="sb", bufs=1))
        dpool = ctx.enter_context(tc.tile_pool(name="diag", bufs=3))
        opool = ctx.enter_context(tc.tile_pool(name="out", bufs=3))
        ps_a = ctx.enter_context(tc.tile_pool(name="ps_a", bufs=2, space="PSUM"))
        ps_qk = ctx.enter_context(tc.tile_pool(name="ps_qk", bufs=2, space="PSUM"))

        # ---- small copy DMAs first (before any xbar-transpose traffic) ----
        wt_sb = cpool.tile([128, E], BF16, tag="wt")
        nc.sync.dma_start(
            wt_sb[:].rearrange("ei (c p) -> ei c p", p=128),
            w[:].rearrange("c ei p -> ei c p"),
        )
        zero_col = cpool.tile([128, 1], F32, tag="zerocol")
        nc.gpsimd.memset(zero_col[:], 0.0)
        bias_sb = cpool.tile([128, 1], F32, tag="bias")
        nc.sync.dma_start(bias_sb[:], wbias[:])
        kbias_sb = cpool.tile([64, 1], F32, tag="kbias")
        nc.sync.dma_start(kbias_sb[:], wbias[64:128, :])
        prior16 = cpool.tile([16, 128], F32, tag="prior16")
        nc.sync.dma_start(prior16[:], priorb[:])
        # ---- bf16 x, transposed by the DMA xbar straight from DRAM ----
        xtiles = []
        for g in range(2):
            xt_sb = xtpool.tile([128, 8192], BF16, tag="xt", bufs=2, name="xt%d" % g)
            nc.sync.dma_start_transpose(
                xt_sb[:].rearrange("p (c s) -> p c s", s=1024),
                xb[1024 * g : 1024 * (g + 1), :].rearrange("s (c p) -> s c p", p=128),
            )
            xtiles.append(xt_sb)

        # ---- constants (gpsimd; overlap the DMAs) ---------------------
        ident = cpool.tile([128, 128], F32, tag="ident")
        make_identity(nc, ident[:])
        ones128 = cpool.tile([128, 128], F32, tag="ones128")
        nc.gpsimd.memset(ones128[:], 1.0)
        triU = cpool.tile([128, 128], F32, tag="triU")  # 1 where k < m (strict upper)
        make_upper_triangular(nc, triU[:], val=1.0, diag=False)
        triL = cpool.tile([128, 128], F32, tag="triL")  # strict lower
        make_lower_triangular(nc, triL[:], val=1.0, diag=False)
        triLE = cpool.tile([128, 128], mybir.dt.int8, tag="triLE")  # lower incl diag
        make_lower_triangular(nc, triLE[:], val=1, diag=True)

        # ---- per-chunk: qkT + kT matmuls, then pipelined kd/pr/prT ----
        qkT = spool.tile([128, S], F32, tag="qkT")
        kT = spool.tile([64, S], F32, tag="kT")
        kd = spool.tile([64, S + 128], F32, tag="kd")
        pr = spool.tile([64, S + 128], F32, tag="pr")
        nc.gpsimd.memset(kd[:, 0:1], 0.0)
        nc.gpsimd.memset(kd[:, S - 1 : S + 128], 0.0)
        nc.gpsimd.memset(pr[:, 0:1], 0.0)
        nc.gpsimd.memset(pr[:, S - 1 : S + 128], 0.0)
        # boundary scores: D[0]=+BIG, D[2047]=-BIG materialize via pr columns
        # (reduction over j picks them up; DN gets D[2047] via its shifted view)
        nc.gpsimd.memset(pr[0:1, 0:1], BIG)
        nc.gpsimd.memset(pr[0:1, S - 1 : S], -BIG)
        prT_D = ps_a.tile([128, 1024], F32, tag="ps_a")
        prT_N = ps_a.tile([128, 1024], F32, tag="ps_a")
        Dcol = spool.tile([128, 16], F32, tag="Dcol")
        DN = spool.tile([128, 16], F32, tag="DN")

        def trD(cc):
            nc.tensor.transpose(
                prT_D[:, 64 * cc : 64 * (cc + 1)],
                pr[:, 128 * cc : 128 * cc + 128],
                ident[0:64, 0:64],
            )

        def trN(cc):
            nc.tensor.transpose(
                prT_N[:, 64 * cc : 64 * (cc + 1)],
                pr[:, 128 * cc + 1 : 128 * cc + 129],
                ident[0:64, 0:64],
            )

        def chunk_tail(c):
            # kd/pr for span owned by chunk c (needs kT up to 512c+512)
            lo = max(1, 512 * c)
            hi = min(S - 1, 512 * (c + 1))
            nc.gpsimd.tensor_sub(kd[:, lo:hi], kT[:, lo + 1 : hi + 1], kT[:, lo - 1 : hi - 1])
            nc.vector.tensor_mul(pr[:, lo:hi], qkT[0:64, lo:hi], kd[:, lo:hi])
            for t in range(4):
                trD(4 * c + t)
            # N-blocks lag by one: block 4c+3 needs pr col 512(c+1)
            nlo, nhi = max(0, 4 * c - 1), 4 * c + 3
            for cc in range(nlo, nhi):
                trN(cc)
            nc.vector.reduce_sum(
                DN[:, nlo:nhi],
                prT_N[:, 64 * nlo : 64 * nhi].rearrange("p (c k) -> p c k", k=64),
                axis=mybir.AxisListType.X,
            )
            nc.vector.reduce_sum(
                Dcol[:, 4 * c : 4 * (c + 1)],
                prT_D[:, 256 * c : 256 * (c + 1)].rearrange("p (c k) -> p c k", k=64),
                axis=mybir.AxisListType.X,
            )

        for sc in range(4):
            qp = ps_qk.tile([128, 1024], F32, tag="ps_qk", name="qp%d" % sc)
            qkp = qp[:, 0:512]
            kp = qp[0:64, 512:1024]
            for e in range(8):
                nc.tensor.matmul(
                    qkp,
                    wt_sb[:, 128 * e : 128 * (e + 1)],
                    xtiles[sc // 2][:, 1024 * e + 512 * (sc % 2) : 1024 * e + 512 * (sc % 2) + 512],
                    start=(e == 0),
                    stop=(e == 7),
                )
            for e in range(8):
                nc.tensor.matmul(
                    kp,
                    wt_sb[:, 128 * e + 64 : 128 * (e + 1)],
                    xtiles[sc // 2][:, 1024 * e + 512 * (sc % 2) : 1024 * e + 512 * (sc % 2) + 512],
                    start=(e == 0),
                    stop=(e == 7),
                )
            nc.scalar.activation(
                qkT[:, 512 * sc : 512 * (sc + 1)], qkp, AF.Identity, bias=bias_sb[:]
            )
            nc.vector.tensor_scalar_add(kT[:, 512 * sc : 512 * (sc + 1)], kp, kbias_sb[:])
            if sc > 0:
                chunk_tail(sc - 1)
        chunk_tail(3)
        trN(15)
        nc.vector.reduce_sum(
            DN[:, 15:16],
            prT_N[:, 960:1024].rearrange("p (c k) -> p c k", k=64),
            axis=mybir.AxisListType.X,
        )

        # ---- scalar chain on (128, 16) col tiles -----------------------
        def ctile(tag):
            return spool.tile([128, 16], F32, tag=tag, name=tag)

        a = ctile("ca")
        nc.scalar.activation(a[:], Dcol[:], AF.Exp, bias=zero_col[:], scale=-1.0 / 1024.0)
        bN = ctile("cb")
        nc.scalar.activation(bN[:], DN[:], AF.Exp, bias=zero_col[:], scale=1.0 / 1024.0)
        A = ctile("cA")
        nc.vector.tensor_scalar_add(A[:], a[:], 1.0)
        Bt = ctile("cB")
        nc.vector.tensor_scalar_add(Bt[:], bN[:], 1.0)
        Pp = ctile("cP")
        nc.vector.tensor_mul(Pp[:], A[:], Bt[:])
        gg = ctile("cgg")
        nc.vector.reciprocal(gg[:], Pp[:])
        uu = ctile("cuu")
        nc.vector.tensor_scalar_add(uu[:], gg[:], 1e-6)
        # prob0 = sqrt(u) via rsqrt bit-trick seed + 3 Newton steps (DVE only)
        yb = ctile("cyb")
        nc.vector.tensor_scalar(
            yb[:].bitcast(I32), uu[:].bitcast(I32), 1, None,
            op0=mybir.AluOpType.arith_shift_right,
        )
        nc.vector.tensor_scalar(
            yb[:].bitcast(I32), yb[:].bitcast(I32), -1, 0x5F3759DF,
            op0=mybir.AluOpType.mult, op1=mybir.AluOpType.add,
        )
        tmp1 = ctile("ctmp1")
        for _ in range(3):
            nc.vector.tensor_mul(tmp1[:], yb[:], yb[:])
            nc.vector.tensor_mul(tmp1[:], tmp1[:], uu[:])
            nc.vector.tensor_scalar(
                tmp1[:], tmp1[:], -0.5, 1.5,
                op0=mybir.AluOpType.mult, op1=mybir.AluOpType.add,
            )
            nc.vector.tensor_mul(yb[:], yb[:], tmp1[:])
        prob0 = ctile("cprob0")
        nc.vector.tensor_mul(prob0[:], uu[:], yb[:])

        prp = ps_a.tile([128, 1024], F32, tag="ps_a")
        # transpose of (16,128) -> (128,16)
        nc.tensor.transpose(prp[:, 0:16], prior16[:], ident[0:16, 0:16])
        prior_col = ctile("cprior")
        nc.vector.tensor_copy(prior_col[:], prp[:, 0:16])

        pm = ctile("cpm")
        nc.vector.tensor_mul(pm[:], prior_col[:], prob0[:])
        ps_ = ctile("cps")
        nc.vector.tensor_add(ps_[:], prior_col[:], prob0[:])
        nb = ctile("cnb")
        nc.vector.tensor_sub(nb[:], ps_[:], pm[:])
        logp = ctile("clogp")
        nc.scalar.activation(logp[:], nb[:], AF.Ln, bias=zero_col[:])

        # neighbor_attn out: (128,16) -> (16,128) -> DRAM
        nbp = ps_a.tile([128, 1024], F32, tag="ps_a")
        nc.tensor.transpose(nbp[0:16, 0:128], nb[:], ident[:])
        nbT = spool.tile([16, 128], F32, tag="nbT")
        nc.vector.tensor_copy(nbT[:], nbp[0:16, 0:128])
        nc.sync.dma_start(outn[:], nbT[:])

        # ---- cumsum (exclusive) ---------------------------------------
        t0 = ctile("sc0")
        nc.gpsimd.memset(t0[:, 0:1], 0.0)
        nc.vector.tensor_copy(t0[:, 1:16], logp[:, 0:15])
        tsc = ctile("tsc")
        nc.vector.tensor_tensor_scan(
            tsc[:], t0[:], t0[:], 0.0,
            op0=mybir.AluOpType.add, op1=mybir.AluOpType.bypass,
        )
        t0 = tsc
        csp = ps_a.tile([128, 1024], F32, tag="ps_a")
        nc.tensor.matmul(csp[:, 0:16], triU[:], logp[:], start=True, stop=False)
        nc.tensor.matmul(csp[:, 0:16], ones128[:], t0[:], start=False, stop=True)
        cs_col = ctile("cs")
        nc.vector.tensor_copy(cs_col[:], csp[:, 0:16])
        negcs = ctile("negcs")
        nc.vector.tensor_scalar_mul(negcs[:], cs_col[:], -1.0)

        # ---- cs broadcast (128, 2048): row j -> cs[j] ------------------
        csb = spool.tile([128, S], F32, tag="csb")
        for cc in range(2):
            pb = ps_a.tile([128, 1024], F32, tag="ps_a", name="pb%d" % cc)
            for k in range(8):
                c = 8 * cc + k
                nc.tensor.transpose(
                    pb[:, 128 * k : 128 * (k + 1)],
                    cs_col[:, c : c + 1].broadcast_to((128, 128)),
                    ident[:],
                )
            nc.vector.tensor_copy(csb[:, 1024 * cc : 1024 * (cc + 1)], pb[:])

        # ---- big exp stage: two row-blocks per out tile ----------------
        for rp in range(8):
            ot = opool.tile([128, 2 * S], F32, tag="out", name="ot%d" % rp)
            for h in range(2):
                r = 2 * rp + h
                j0 = 128 * r
                j1 = j0 + 128
                o = ot[:, 2048 * h : 2048 * (h + 1)]
                # upper incl. diagonal block: exp(cs[j] - cs[i])
                nc.scalar.activation(
                    o[:, j0:S], csb[:, j0:S], AF.Exp, bias=negcs[:, r : r + 1], scale=1.0
                )
                if r > 0:
                    # strict lower: exp(cs[i] - cs[j])
                    nc.scalar.activation(
                        o[:, 0:j0], csb[:, 0:j0], AF.Exp,
                        bias=cs_col[:, r : r + 1], scale=-1.0,
                    )
                rec = dpool.tile([128, 128], F32, tag="rec", name="rec%d" % r)
                nc.vector.reciprocal(rec[:], o[:, j0:j1])
                tl = dpool.tile([128, 128], F32, tag="tl", name="tl%d" % r)
                nc.vector.tensor_mul(tl[:], rec[:], triL[:])
                nc.vector.copy_predicated(o[:, j0:j1], triLE[:], tl[:])
            nc.sync.dma_start(
                outc[256 * rp : 256 * (rp + 1), :].rearrange("(b p) j -> p b j", p=128),
                ot[:].rearrange("p (b j) -> p b j", j=S),
            )


_NC_CACHE = {}


def _get_nc():
    if "nc" not in _NC_CACHE:
        _NC_CACHE["nc"] = build_nc()
    return _NC_CACHE["nc"]


def make_in_maps(context, prior, proj_weight, proj_bias):
    context = np.asarray(context, dtype=np.float32)
    prior = np.asarray(prior, dtype=np.float32)
    proj_weight = np.ascontiguousarray(np.asarray(proj_weight, dtype=np.float32))
    proj_bias = np.ascontiguousarray(
        np.asarray(proj_bias, dtype=np.float32).reshape(128, 1)
    )
    in_maps = []
    for b in range(B):
        in_maps.append(
            {
                "xb": np.ascontiguousarray(context[:, b, :]).astype(BFH),
                "w": np.ascontiguousarray(proj_weight.T.astype(BFH).reshape(8, 128, 128)),
                "wbias": proj_bias,
                "priorb": np.ascontiguousarray(prior[b].reshape(16, 128)),
            }
        )
    return in_maps


def kernel(context, prior, proj_weight, proj_bias):
    nc = _get_nc()
    in_maps = make_in_maps(context, prior, proj_weight, proj_bias)
    res = run_bass_kernel_spmd(nc, in_maps, list(range(B)))
    constituent = np.stack([res.results[b]["outc"] for b in range(B)], axis=0)
    neighbor = np.stack([res.results[b]["outn"].reshape(S) for b in range(B)], axis=0)
    return constituent, neighbor


# revision 25
# speedup vs baseline: 1.0466x; 1.0466x over previous
"""ConstituentAttention Trainium2 kernel.

Data-parallel over batch: core b handles batch b (B == 8 == n_cores).

Math (per batch, S=2048, E=1024, P=64):
  qkT[p, s] = sum_e W[p, e] x[s, e] + bias[p]         (p in [0,128): q rows 0..63, k rows 64..127)
  D_raw[s]  = sum_j q[j, s] * (k[j, s+1] - k[j, s-1])  == 1024*(score0[s] - score1[s])
  with D_raw[0] := +BIG, D_raw[2047] := -BIG (softmax -inf boundary handling),
  p0[s] = sigmoid(D[s]), shifted p1[s] = sigmoid(-D[s+1]), so
  g[s]  = 1 / ((1 + e^{-D[s]}) (1 + e^{+D[s+1]}))
  prob0 = sqrt(g + 1e-6) = exp(0.5*(ln(1e-6*P + 1) - ln(P))),  P = (1+e^-D)(1+e^+Dnext)
  neighbor = prior + prob0 - prior*prob0
  cs[t] = sum_{m<t} ln(neighbor[m])
  out[i, j] = exp(cs[max(i,j)] - cs[min(i,j)]),  0 on diag.
"""

import sys

sys.path.insert(0, "/opt/trn_rl_repo")

import ml_dtypes
import numpy as np

BFH = ml_dtypes.bfloat16

import concourse.bass as bass
import concourse.bacc as bacc
import concourse.tile as tile
from concourse import mybir
from concourse.bass_utils import run_bass_kernel_spmd
from concourse.masks import (
    make_identity,
    make_lower_triangular,
    make_upper_triangular,
)

F32 = mybir.dt.float32
F32R = mybir.dt.float32r
BF16 = mybir.dt.bfloat16
I32 = mybir.dt.int32
AF = mybir.ActivationFunctionType

S = 2048
E = 1024
B = 8
NCHUNK = 16  # S / 128
# Raw-score boundary magnitude; Exp scale 1/1024 maps it to +-30.
# 30 is chosen so 1+e^-30 == 1.0 exactly in f32 (p0[0] == 1) while
# P = (1+e^30)*2 ~ 4e13 stays inside ACT Ln's 2^64 domain and
# 1e-6*P >> 1 makes prob0[2047] -> sqrt(1e-6) as in the reference.
BIG = 30.0 * 1024.0


def build_nc(n_cores=8):
    nc = bacc.Bacc("TRN2", target_bir_lowering=False, debug=False, num_devices=n_cores, enable_partition_id=False)

    xb = nc.dram_tensor("xb", [S, E], BF16, kind="ExternalInput")
    w = nc.dram_tensor("w", [8, 128, 128], BF16, kind="ExternalInput")
    wbias = nc.dram_tensor("wbias", [128, 1], F32, kind="ExternalInput")
    priorb = nc.dram_tensor("priorb", [16, 128], F32, kind="ExternalInput")
    outc = nc.dram_tensor("outc", [S, S], F32, kind="ExternalOutput")
    outn = nc.dram_tensor("outn", [16, 128], F32, kind="ExternalOutput")

    with tile.TileContext(nc) as tc:
        build_tile_kernel(nc, tc, xb, w, wbias, priorb, outc, outn)
    nc.compile()
    return nc


def build_tile_kernel(nc, tc, xb, w, wbias, priorb, outc, outn):
    from contextlib import ExitStack

    ctx = ExitStack()
    with ctx:
        cpool = ctx.enter_context(tc.tile_pool(name="consts", bufs=1))
        xtpool = ctx.enter_context(tc.tile_pool(name="xt", bufs=2))
        spool = ctx.enter_context(tc.tile_pool(name="sb", bufs=1))
        dpool = ctx.enter_context(tc.tile_pool(name="diag", bufs=3))
        opool = ctx.enter_context(tc.tile_pool(name="out", bufs=3))
        ps_a = ctx.enter_context(tc.tile_pool(name="ps_a", bufs=2, space="PSUM"))
        ps_qk = ctx.enter_context(tc.tile_pool(name="ps_qk", bufs=2, space="PSUM"))

        # ---- small copy DMAs first (before any xbar-transpose traffic) ----
        wt_sb = cpool.tile([128, E], BF16, tag="wt")
        nc.sync.dma_start(
            wt_sb[:].rearrange("ei (c p) -> ei c p", p=128),
            w[:].rearrange("c ei p -> ei c p"),
        )
        zero_col = cpool.tile([128, 1], F32, tag="zerocol")
        nc.gpsimd.memset(zero_col[:], 0.0)
        bias_sb = cpool.tile([128, 1], F32, tag="bias")
        nc.sync.dma_start(bias_sb[:], wbias[:])
        kbias_sb = cpool.tile([64, 1], F32, tag="kbias")
        nc.sync.dma_start(kbias_sb[:], wbias[64:128, :])
        prior16 = cpool.tile([16, 128], F32, tag="prior16")
        nc.sync.dma_start(prior16[:], priorb[:])
        # ---- bf16 x, transposed by the DMA xbar straight from DRAM ----
        xtiles = []
        for g in range(4):
            xt_sb = xtpool.tile([128, 4096], BF16, tag="xt", bufs=4, name="xt%d" % g)
            nc.sync.dma_start_transpose(
                xt_sb[:].rearrange("p (c s) -> p c s", s=512),
                xb[512 * g : 512 * (g + 1), :].rearrange("s (c p) -> s c p", p=128),
            )
            xtiles.append(xt_sb)

        # ---- constants (gpsimd; overlap the DMAs) ---------------------
        ident = cpool.tile([128, 128], F32, tag="ident")
        make_identity(nc, ident[:])
        ones128 = cpool.tile([128, 128], F32, tag="ones128")
        nc.gpsimd.memset(ones128[:], 1.0)
        triU = cpool.tile([128, 128], F32, tag="triU")  # 1 where k < m (strict upper)
        make_upper_triangular(nc, triU[:], val=1.0, diag=False)
        triL = cpool.tile([128, 128], F32, tag="triL")  # strict lower
        make_lower_triangular(nc, triL[:], val=1.0, diag=False)
        triLE = cpool.tile([128, 128], mybir.dt.int8, tag="triLE")  # lower incl diag
        make_lower_triangular(nc, triLE[:], val=1, diag=True)

        # ---- per-chunk: qkT + kT matmuls, then pipelined kd/pr/prT ----
        qkT = spool.tile([128, S], F32, tag="qkT")
        kT = spool.tile([64, S], F32, tag="kT")
        kd = spool.tile([64, S + 128], F32, tag="kd")
        pr = spool.tile([64, S + 128], F32, tag="pr")
        nc.gpsimd.memset(kd[:, 0:1], 0.0)
        nc.gpsimd.memset(kd[:, S - 1 : S + 128], 0.0)
        nc.gpsimd.memset(pr[:, 0:1], 0.0)
        nc.gpsimd.memset(pr[:, S - 1 : S + 128], 0.0)
        # boundary scores: D[0]=+BIG, D[2047]=-BIG materialize via pr columns
        # (reduction over j picks them up; DN gets D[2047] via its shifted view)
        nc.gpsimd.memset(pr[0:1, 0:1], BIG)
        nc.gpsimd.memset(pr[0:1, S - 1 : S], -BIG)
        prT_D = ps_a.tile([128, 1024], F32, tag="ps_a")
        prT_N = ps_a.tile([128, 1024], F32, tag="ps_a")
        Dcol = spool.tile([128, 16], F32, tag="Dcol")
        DN = spool.tile([128, 16], F32, tag="DN")

        def trD(cc):
            nc.tensor.transpose(
                prT_D[:, 64 * cc : 64 * (cc + 1)],
                pr[:, 128 * cc : 128 * cc + 128],
                ident[0:64, 0:64],
            )

        def trN(cc):
            nc.tensor.transpose(
                prT_N[:, 64 * cc : 64 * (cc + 1)],
                pr[:, 128 * cc + 1 : 128 * cc + 129],
                ident[0:64, 0:64],
            )

        BOUNDS = [0, 512, 1024, 1536, 1920, 2048]

        def chunk_tail(c):
            # kd/pr for span owned by chunk c (needs kT up to BOUNDS[c+1])
            s0, s1 = BOUNDS[c], BOUNDS[c + 1]
            lo = max(1, s0)
            hi = min(S - 1, s1)
            nc.vector.tensor_sub(kd[:, lo:hi], kT[:, lo + 1 : hi + 1], kT[:, lo - 1 : hi - 1])
            nc.vector.tensor_mul(pr[:, lo:hi], qkT[0:64, lo:hi], kd[:, lo:hi])
            b0, b1 = s0 // 128, s1 // 128
            for cc in range(b0, b1):
                trD(cc)
            # N-blocks lag by one: block b needs pr col 128b+128
            nlo, nhi = max(0, b0 - 1), b1 - 1
            for cc in range(nlo, nhi):
                trN(cc)
            if nhi > nlo:
                nc.vector.reduce_sum(
                    DN[:, nlo:nhi],
                    prT_N[:, 64 * nlo : 64 * nhi].rearrange("p (c k) -> p c k", k=64),
                    axis=mybir.AxisListType.X,
                )
            nc.vector.reduce_sum(
                Dcol[:, b0:b1],
                prT_D[:, 64 * b0 : 64 * b1].rearrange("p (c k) -> p c k", k=64),
                axis=mybir.AxisListType.X,
            )

        for sc in range(5):
            s0, s1 = BOUNDS[sc], BOUNDS[sc + 1]
            wdt = s1 - s0
            g, off = s0 // 512, s0 % 512
            qp = ps_qk.tile([128, 1024], F32, tag="ps_qk", name="qp%d" % sc)
            qkp = qp[:, 0:wdt]
            kp = qp[0:64, 512 : 512 + wdt]
            for e in range(8):
                nc.tensor.matmul(
                    qkp,
                    wt_sb[:, 128 * e : 128 * (e + 1)],
                    xtiles[g][:, 512 * e + off : 512 * e + off + wdt],
                    start=(e == 0),
                    stop=(e == 7),
                )
            for e in range(8):
                nc.tensor.matmul(
                    kp,
                    wt_sb[:, 128 * e + 64 : 128 * (e + 1)],
                    xtiles[g][:, 512 * e + off : 512 * e + off + wdt],
                    start=(e == 0),
                    stop=(e == 7),
                )
            nc.scalar.activation(qkT[:, s0:s1], qkp, AF.Identity, bias=bias_sb[:])
            nc.vector.tensor_scalar_add(kT[:, s0:s1], kp, kbias_sb[:])
            if sc > 0:
                chunk_tail(sc - 1)
        chunk_tail(4)
        trN(15)
        nc.vector.reduce_sum(
            DN[:, 15:16],
            prT_N[:, 960:1024].rearrange("p (c k) -> p c k", k=64),
            axis=mybir.AxisListType.X,
        )

        # ---- scalar chain on (128, 16) col tiles -----------------------
        def ctile(tag):
            return spool.tile([128, 16], F32, tag=tag, name=tag)

        a = ctile("ca")
        nc.scalar.activation(a[:], Dcol[:], AF.Exp, bias=zero_col[:], scale=-1.0 / 1024.0)
        bN = ctile("cb")
        nc.scalar.activation(bN[:], DN[:], AF.Exp, bias=zero_col[:], scale=1.0 / 1024.0)
        A = ctile("cA")
        nc.vector.tensor_scalar_add(A[:], a[:], 1.0)
        Bt = ctile("cB")
        nc.vector.tensor_scalar_add(Bt[:], bN[:], 1.0)
        Pp = ctile("cP")
        nc.vector.tensor_mul(Pp[:], A[:], Bt[:])
        gg = ctile("cgg")
        nc.vector.reciprocal(gg[:], Pp[:])
        uu = ctile("cuu")
        nc.vector.tensor_scalar_add(uu[:], gg[:], 1e-6)
        # prob0 = sqrt(u) via rsqrt bit-trick seed + 3 Newton steps (DVE only)
        yb = ctile("cyb")
        nc.vector.tensor_scalar(
            yb[:].bitcast(I32), uu[:].bitcast(I32), 1, None,
            op0=mybir.AluOpType.arith_shift_right,
        )
        nc.vector.tensor_scalar(
            yb[:].bitcast(I32), yb[:].bitcast(I32), -1, 0x5F3759DF,
            op0=mybir.AluOpType.mult, op1=mybir.AluOpType.add,
        )
        tmp1 = ctile("ctmp1")
        for _ in range(3):
            nc.vector.tensor_mul(tmp1[:], yb[:], yb[:])
            nc.vector.tensor_mul(tmp1[:], tmp1[:], uu[:])
            nc.vector.tensor_scalar(
                tmp1[:], tmp1[:], -0.5, 1.5,
                op0=mybir.AluOpType.mult, op1=mybir.AluOpType.add,
            )
            nc.vector.tensor_mul(yb[:], yb[:], tmp1[:])
        prob0 = ctile("cprob0")
        nc.vector.tensor_mul(prob0[:], uu[:], yb[:])

        prp = ps_a.tile([128, 1024], F32, tag="ps_a")
        # transpose of (16,128) -> (128,16)
        nc.tensor.transpose(prp[:, 0:16], prior16[:], ident[0:16, 0:16])
        prior_col = ctile("cprior")
        nc.vector.tensor_copy(prior_col[:], prp[:, 0:16])

        pm = ctile("cpm")
        nc.vector.tensor_mul(pm[:], prior_col[:], prob0[:])
        ps_ = ctile("cps")
        nc.vector.tensor_add(ps_[:], prior_col[:], prob0[:])
        nb = ctile("cnb")
        nc.vector.tensor_sub(nb[:], ps_[:], pm[:])
        logp = ctile("clogp")
        nc.scalar.activation(logp[:], nb[:], AF.Ln, bias=zero_col[:])

        # neighbor_attn out: (128,16) -> (16,128) -> DRAM
        nbp = ps_a.tile([128, 1024], F32, tag="ps_a")
        nc.tensor.transpose(nbp[0:16, 0:128], nb[:], ident[:])
        nbT = spool.tile([16, 128], F32, tag="nbT")
        nc.vector.tensor_copy(nbT[:], nbp[0:16, 0:128])
        nc.sync.dma_start(outn[:], nbT[:])

        # ---- cumsum (exclusive) ---------------------------------------
        t0 = ctile("sc0")
        nc.gpsimd.memset(t0[:, 0:1], 0.0)
        nc.vector.tensor_copy(t0[:, 1:16], logp[:, 0:15])
        tsc = ctile("tsc")
        nc.vector.tensor_tensor_scan(
            tsc[:], t0[:], t0[:], 0.0,
            op0=mybir.AluOpType.add, op1=mybir.AluOpType.bypass,
        )
        t0 = tsc
        csp = ps_a.tile([128, 1024], F32, tag="ps_a")
        nc.tensor.matmul(csp[:, 0:16], triU[:], logp[:], start=True, stop=False)
        nc.tensor.matmul(csp[:, 0:16], ones128[:], t0[:], start=False, stop=True)
        cs_col = ctile("cs")
        nc.vector.tensor_copy(cs_col[:], csp[:, 0:16])
        negcs = ctile("negcs")
        nc.vector.tensor_scalar_mul(negcs[:], cs_col[:], -1.0)

        # ---- cs broadcast (128, 2048): row j -> cs[j] ------------------
        csb = spool.tile([128, S], F32, tag="csb")
        for cc in range(2):
            pb = ps_a.tile([128, 1024], F32, tag="ps_a", name="pb%d" % cc)
            for k in range(8):
                c = 8 * cc + k
                nc.tensor.transpose(
                    pb[:, 128 * k : 128 * (k + 1)],
                    cs_col[:, c : c + 1].broadcast_to((128, 128)),
                    ident[:],
                )
            nc.vector.tensor_copy(csb[:, 1024 * cc : 1024 * (cc + 1)], pb[:])

        # ---- big exp stage: two row-blocks per out tile ----------------
        for rp in range(8):
            ot = opool.tile([128, 2 * S], F32, tag="out", name="ot%d" % rp)
            for h in range(2):
                r = 2 * rp + h
                j0 = 128 * r
                j1 = j0 + 128
                o = ot[:, 2048 * h : 2048 * (h + 1)]
                # upper incl. diagonal block: exp(cs[j] - cs[i])
                nc.scalar.activation(
                    o[:, j0:S], csb[:, j0:S], AF.Exp, bias=negcs[:, r : r + 1], scale=1.0
                )
                if r > 0:
                    # strict lower: exp(cs[i] - cs[j])
                    nc.scalar.activation(
                        o[:, 0:j0], csb[:, 0:j0], AF.Exp,
                        bias=cs_col[:, r : r + 1], scale=-1.0,
                    )
                rec = dpool.tile([128, 128], F32, tag="rec", name="rec%d" % r)
                nc.vector.reciprocal(rec[:], o[:, j0:j1])
                tl = dpool.tile([128, 128], F32, tag="tl", name="tl%d" % r)
                nc.vector.tensor_mul(tl[:], rec[:], triL[:])
                nc.vector.copy_predicated(o[:, j0:j1], triLE[:], tl[:])
            nc.sync.dma_start(
                outc[256 * rp : 256 * (rp + 1), :].rearrange("(b p) j -> p b j", p=128),
                ot[:].rearrange("p (b j) -> p b j", j=S),
            )


_NC_CACHE = {}


def _get_nc():
    if "nc" not in _NC_CACHE:
        _NC_CACHE["nc"] = build_nc()
    return _NC_CACHE["nc"]


def make_in_maps(context, prior, proj_weight, proj_bias):
    context = np.asarray(context, dtype=np.float32)
    prior = np.asarray(prior, dtype=np.float32)
    proj_weight = np.ascontiguousarray(np.asarray(proj_weight, dtype=np.float32))
    proj_bias = np.ascontiguousarray(
        np.asarray(proj_bias, dtype=np.float32).reshape(128, 1)
    )
    in_maps = []
    for b in range(B):
        in_maps.append(
            {
                "xb": np.ascontiguousarray(context[:, b, :]).astype(BFH),
                "w": np.ascontiguousarray(proj_weight.T.astype(BFH).reshape(8, 128, 128)),
                "wbias": proj_bias,
                "priorb": np.ascontiguousarray(prior[b].reshape(16, 128)),
            }
        )
    return in_maps


def kernel(context, prior, proj_weight, proj_bias):
    nc = _get_nc()
    in_maps = make_in_maps(context, prior, proj_weight, proj_bias)
    res = run_bass_kernel_spmd(nc, in_maps, list(range(B)))
    constituent = np.stack([res.results[b]["outc"] for b in range(B)], axis=0)
    neighbor = np.stack([res.results[b]["outn"].reshape(S) for b in range(B)], axis=0)
    return constituent, neighbor
